# revision 39
# baseline (speedup 1.0000x reference)
"""Causal self-attention (dense transformer) on 8 trn2 NeuronCores.

Reference semantics (note the headless reshape):
  x_proj = x @ Wqkv + bqkv                     # [B, T, 3C]
  q = x_proj[:, :, :C].reshape(B, H, T, hd)    # direct reshape, no transpose!
Because of the direct reshape, head h consumes the contiguous row block
x_proj[b, h*128:(h+1)*128, :] reinterpreted as [T, hd].  So sharding by
(batch, head-group) makes QKV projection + attention fully core-local;
only the output projection is a row-parallel partial sum, reduced on host.

Shapes (hardcoded): B=2, T=2048, C=1024, n_head=16, hd=64, 8 cores.
Core c: batch b=c//4, quarter q=c%4 -> x rows [512q, 512q+512), heads 4q..4q+3.

Device layout tricks:
- Q,K columns of x_proj computed in TRANSPOSED orientation (lhsT=Wqkv tile,
  rhs=x^T tile): the per-head Q^T/K^T [hd, T] layouts fall out of the PSUM
  eviction with a stride-16 destination AP (no PE transposes at all).  Bias
  is folded into the eviction (tensor_scalar add, per-partition scalar).
- V columns computed in natural orientation, bounced through a DRAM scratch
  and gathered back as [s, hd] tiles (re-partition); a ones column is
  appended so P@[V|1] also yields the softmax denominator row.
- exp on ACT reads 2-bank PSUM S^T tiles directly, 1/sqrt(hd) folded into
  the activation scale; causal masking via gpsimd affine_select (in-place).
- softmax normalization: reciprocal of denom row, broadcast via a K=1 PE
  outer product, multiply on DVE during Y eviction.
- all matmul operands are float32r (~13-bit mantissa, 1 cyc/row on PE).
"""

import os

import numpy as np

os.environ.setdefault("NEURON_RT_RESET_CORES", "1")

import concourse.bacc as bacc
import concourse.mybir as mybir
import concourse.tile as tile
from concourse.bass_utils import run_bass_kernel_spmd

dt = mybir.dt
AF = mybir.ActivationFunctionType
OP = mybir.AluOpType

B, T, C = 2, 2048, 1024
NH, HD = 16, 64
N_CORES = 8
HPC = 4          # heads per core
RPC = 512        # x rows per core
SCALE = 1.0 / 8.0   # 1/sqrt(hd), folded into the exp activation


def build_program():
    nc = bacc.Bacc("TRN2", target_bir_lowering=False, debug=False,
                   num_devices=N_CORES)

    # ---- DRAM I/O (per core) ----
    xT = nc.dram_tensor("xT", [128, 8, RPC], dt.float32r, kind="ExternalInput")
    wq = nc.dram_tensor("wq", [16, 128, 8 * 128], dt.float32r, kind="ExternalInput")
    wv = nc.dram_tensor("wv", [2, 128, 8 * 512], dt.float32r, kind="ExternalInput")
    bqk = nc.dram_tensor("bqk", [1, 2048], dt.float32r, kind="ExternalInput")
    bv = nc.dram_tensor("bv", [1, 1024], dt.float32r, kind="ExternalInput")
    wp = nc.dram_tensor("wp", [128, 2 * 1024], dt.float32r, kind="ExternalInput")
    bp = nc.dram_tensor("bp", [128, 1024], dt.float32, kind="ExternalInput")
    ones512 = nc.dram_tensor("ones512", [1, 512], dt.float32r, kind="ExternalInput")
    ones16 = nc.dram_tensor("ones16", [128, 16], dt.float32r, kind="ExternalInput")
    out_d = nc.dram_tensor("out", [T, C], dt.float32, kind="ExternalOutput")

    with tile.TileContext(nc) as tc:
        with tc.tile_pool(name="persist", bufs=1) as pp, \
             tc.tile_pool(name="drampool", bufs=1, space="DRAM") as dp:
            vscr = [dp.tile([128, 1024], dt.float32r, tag=f"vscr{h}",
                            name=f"vscr{h}") for h in range(HPC)]

            xt = pp.tile([128, 8, RPC], dt.float32r, tag="xt")
            bqk_sb = pp.tile([1, 2048], dt.float32r, tag="bqk")
            bv_sb = pp.tile([1, 1024], dt.float32r, tag="bv")
            onesr = pp.tile([1, 512], dt.float32r, tag="onesr")
            ones16_sb = pp.tile([128, 16], dt.float32r, tag="ones16")
            wp_sb = pp.tile([128, 2, 1024], dt.float32r, tag="wp")
            bp_sb = pp.tile([128, 1024], dt.float32, tag="bp")

            qt_all = pp.tile([64, HPC * T], dt.float32r, tag="qt_all")
            kt_all = pp.tile([64, HPC * T], dt.float32r, tag="kt_all")
            vn = [pp.tile([128, 16 * 65], dt.float32r, tag=f"vn{h}", name=f"vn{h}")
                  for h in range(HPC)]            # per-head [V | 1] s-tiles
            yt = [pp.tile([128, T], dt.float32r, tag=f"yt{p}", name=f"yt{p}")
                  for p in range(2)]

            # attention pools opened early: first S/exp groups are hoisted
            # into phase 1 so ACT warms up while PE finishes the V part
            with tc.tile_pool(name="ptpool", bufs=10) as ptp, \
                 tc.tile_pool(name="ps2", bufs=2, space="PSUM") as ps2:

                def emit_sexp(h, j, sp):
                    """S^T matmuls for an s-pair + exp + causal mask."""
                    ssp = ps2.tile([128, 1024], dt.float32, tag="spsum",
                                   name=f"ssp{h}{j}{sp}")
                    for half in range(2):
                        i = 2 * sp + half
                        nc.tensor.matmul(
                            ssp[:, 512 * half:512 * (half + 1)],
                            kt_all[:, T * h + 128 * i:T * h + 128 * (i + 1)],
                            qt_all[:, T * h + 512 * j:T * h + 512 * (j + 1)],
                            start=True, stop=True)
                    pt = ptp.tile([128, 1024], dt.float32r, tag="pt",
                                  name=f"pt{h}{j}{sp}")
                    nc.scalar.activation(pt[:], ssp[:], AF.Exp, scale=SCALE)
                    for half in range(2):
                        i = 2 * sp + half
                        if i >= 4 * j:  # diagonal band: causal mask
                            nc.gpsimd.affine_select(
                                out=pt[:, 512 * half:512 * (half + 1)],
                                in_=pt[:, 512 * half:512 * (half + 1)],
                                compare_op=OP.is_ge, fill=0.0,
                                base=512 * j - 128 * i,
                                channel_multiplier=-1,
                                pattern=[[1, 512]])
                    return pt

                # ================= Phase 1: QKV projection =================
                with tc.tile_pool(name="wstream", bufs=2) as ws, \
                     tc.tile_pool(name="ps1", bufs=2, space="PSUM") as ps1:
                    # --- Q,K in transposed orientation: x_proj^T j-tiles ---
                    for m in range(16):
                        wqt = ws.tile([128, 8, 128], dt.float32r, tag="wqt")
                        nc.sync.dma_start(wqt[:], wq[m].rearrange(
                            "p (k j) -> p k j", k=8))
                        if m == 0:
                            for k in range(8):
                                nc.sync.dma_start(xt[:, k, :], xT[:, k, :])
                            nc.sync.dma_start(bqk_sb[:], bqk[:])
                            nc.sync.dma_start(bv_sb[:], bv[:])
                            nc.sync.dma_start(onesr[:], ones512[:])
                            nc.sync.dma_start(ones16_sb[:], ones16[:])
                        ps = ps1.tile([128, RPC], dt.float32, tag="psqk")
                        for k in range(8):
                            nc.tensor.matmul(ps[:], wqt[:, k, :], xt[:, k, :],
                                             start=(k == 0), stop=False)
                        nc.tensor.matmul(ps[:], bqk_sb[:, 128 * m:128 * (m + 1)],
                                         onesr[:, 0:RPC], start=False, stop=True)
                        # evict with bias + stride-16 shuffle into Q^T / K^T
                        # free index = 2048h + 16rh + (gp+par): one strided AP
                        # covers all 4 heads (source free r = 128h + rh aligns)
                        dest = qt_all if m < 8 else kt_all
                        gp = 2 * (m % 8)
                        for par in range(2):
                            nc.scalar.activation(
                                dest[:, gp + par:HPC * T:16],
                                ps[64 * par:64 * par + 64, :],
                                AF.Copy, scale=1.0)

                    # hoisted S/exp for (j=3, h=0,1): keeps ACT busy during V
                    hoisted = {(0, sp): emit_sexp(0, 3, sp) for sp in range(8)}
                    hoisted.update({(1, sp): emit_sexp(1, 3, sp) for sp in range(3)})

                    # --- V in natural orientation -> DRAM scratch ---
                    # (virtual-time delay: let the wq stream own DMA bandwidth
                    # so attention can start as early as possible)
                    tc.tile_set_cur_wait(0.024)
                    for jv in range(2):
                        wvt = ws.tile([128, 8, 512], dt.float32r, tag="wvt",
                                      bufs=1, name=f"wvt{jv}")
                        for kh in range(2):
                            nc.sync.dma_start(
                                wvt[:, 4 * kh:4 * kh + 4, :],
                                wv[jv, :, 2048 * kh:2048 * (kh + 1)].rearrange(
                                    "p (k j) -> p k j", k=4))
                        for h in range(HPC):
                            ps = ps1.tile([128, 512], dt.float32, tag="psv", bufs=2)
                            for k in range(8):
                                nc.tensor.matmul(
                                    ps[:], xt[:, k, 128 * h:128 * (h + 1)],
                                    wvt[:, k, :], start=(k == 0), stop=False)
                            nc.tensor.matmul(ps[:], onesr[:, 0:128],
                                             bv_sb[:, 512 * jv:512 * (jv + 1)],
                                             start=False, stop=True)
                            vsb = ws.tile([128, 512], dt.float32r, tag="vsb",
                                          bufs=1)
                            nc.vector.tensor_copy(vsb[:], ps[:])
                            nc.sync.dma_start(
                                vscr[h][:, 512 * jv:512 * (jv + 1)], vsb[:])

                nc.sync.dma_start(wp_sb[:], wp.rearrange("p (t c) -> p t c", t=2))
                nc.sync.dma_start(bp_sb[:], bp[:])
                tc.tile_set_cur_wait(0.0)

                # --- gather V natural [s, d] + ones cols (one DMA per head:
                # src AP [[1024,8],[64,16],[8192,16],[1,64]] over the flat
                # scratch; dest free dims (i:65-stride, d)) ---
                for h in range(HPC):
                    src_ap = vscr[h][:].rearrange(
                        "(i r) (g d) -> (r g) i d", r=8, d=64)
                    dst_ap = vn[h][:].rearrange("p (i e) -> p i e", e=65)[:, :, 0:64]
                    nc.sync.dma_start(dst_ap, src_ap)
                    nc.sync.dma_start(vn[h][:, 64:16 * 65:65], ones16_sb[:])

                # ===== Phase 2+3: attention (j desc) + fused projection =====
                with tc.tile_pool(name="misc", bufs=2) as mp, \
                     tc.tile_pool(name="osb", bufs=3) as osbp, \
                     tc.tile_pool(name="psy", bufs=2, space="PSUM") as psy, \
                     tc.tile_pool(name="ps3", bufs=2, space="PSUM") as ps3:

                    def emit_pv(h, sp, pt, yps, n_st):
                        for half in range(2):
                            i = 2 * sp + half
                            nc.tensor.matmul(
                                yps[:], vn[h][:, 65 * i:65 * i + 65],
                                pt[:, 512 * half:512 * (half + 1)],
                                start=(i == 0), stop=(i == n_st - 1))

                    def make_norm(h, j, yps):
                        def norm():
                            den = mp.tile([1, 512], dt.float32r, tag="den",
                                          name=f"den{h}{j}")
                            nc.vector.tensor_copy(den[:], yps[64:65, :])
                            rec = mp.tile([1, 512], dt.float32r, tag="rec",
                                          name=f"rec{h}{j}")
                            with nc.allow_low_precision(reason="softmax recip"):
                                nc.vector.reciprocal(rec[:], den[:])
                            bcp = ps3.tile([128, 512], dt.float32, tag="px",
                                           name=f"bcp{h}{j}")[0:64, :]
                            nc.tensor.matmul(bcp[:], onesr[:, 0:64], rec[:],
                                             start=True, stop=True)
                            bcs = mp.tile([64, 512], dt.float32, tag="bcs",
                                          name=f"bcs{h}{j}")
                            nc.vector.tensor_copy(bcs[:], bcp[:])
                            nc.vector.tensor_tensor(
                                yt[h // 2][64 * (h % 2):64 * (h % 2) + 64,
                                           512 * j:512 * (j + 1)],
                                yps[0:64, :], bcs[:], op=OP.mult)
                        return norm

                    def make_proj_one(j, tt, cc, last=False):
                        def proj():
                            if last and (tt + cc) % 2 == 0:
                                pw = ps2.tile([128, 1024], dt.float32,
                                              tag="spsum", name=f"pow{tt}{cc}")
                                po = pw[:, 0:512]
                            else:
                                po = ps3.tile([128, 512], dt.float32,
                                              tag="px", name=f"po{tt}{cc}")
                            nc.tensor.matmul(
                                po[:], yt[0][:, 128 * tt:128 * (tt + 1)],
                                wp_sb[:, 0, 512 * cc:512 * (cc + 1)],
                                start=True, stop=False)
                            nc.tensor.matmul(
                                po[:], yt[1][:, 128 * tt:128 * (tt + 1)],
                                wp_sb[:, 1, 512 * cc:512 * (cc + 1)],
                                start=False, stop=True)
                            ot = osbp.tile([128, 512], dt.float32,
                                           tag="ot", name=f"ot{tt}{cc}")
                            nc.vector.tensor_tensor(
                                ot[:], po[:],
                                bp_sb[:, 512 * cc:512 * (cc + 1)], op=OP.add)
                            nc.sync.dma_start(
                                out_d[128 * tt:128 * (tt + 1),
                                      512 * cc:512 * (cc + 1)], ot[:])
                        return proj

                    pending = []   # small deferred closures, drip-fed
                    for jx, j in enumerate([3, 2, 1, 0]):
                        for h in range(HPC):
                            n_st = 4 * j + 4        # s-tiles needed (causal)
                            yps = psy.tile([65, 512], dt.float32, tag="ypsum",
                                           name=f"yps{h}{j}")
                            prev = None
                            for sp in range(n_st // 2):
                                if jx == 0 and (h, sp) in hoisted:
                                    pt = hoisted[(h, sp)]
                                else:
                                    pt = emit_sexp(h, j, sp)
                                if prev is not None:
                                    psp, pt_prev = prev
                                    emit_pv(h, psp, pt_prev, yps, n_st)
                                if sp >= min(2, n_st // 2 - 1) and pending:
                                    pending.pop(0)()
                                prev = (sp, pt)
                            psp, pt_prev = prev
                            emit_pv(h, psp, pt_prev, yps, n_st)
                            pending.append(make_norm(h, j, yps))
                        for tt in range(4 * j, 4 * j + 4):
                            for cc in range(2):
                                pending.append(
                                    make_proj_one(j, tt, cc, last=(jx == 3)))
                    for fn in pending:
                        fn()
    nc.compile()
    return nc


_NC_CACHE = None


def _get_program():
    global _NC_CACHE
    if _NC_CACHE is None:
        _NC_CACHE = build_program()
    return _NC_CACHE


def _prep_core_inputs(x, Wqkv, bqkv, Wproj, bproj):
    """Build the 8 per-core input dicts (host-side shard + layout prep)."""
    x = np.asarray(x, dtype=np.float32)
    Wqkv = np.ascontiguousarray(np.asarray(Wqkv, dtype=np.float32))
    bqkv = np.asarray(bqkv, dtype=np.float32)
    Wproj = np.asarray(Wproj, dtype=np.float32)
    bproj = np.asarray(bproj, dtype=np.float32)

    wq_np = np.ascontiguousarray(
        Wqkv[:, :2048].reshape(8, 128, 16, 128).transpose(2, 1, 0, 3)
        .reshape(16, 128, 8 * 128))
    wv_np = np.ascontiguousarray(
        Wqkv[:, 2048:].reshape(8, 128, 2, 512).transpose(2, 1, 0, 3)
        .reshape(2, 128, 8 * 512))
    bqk_np = np.ascontiguousarray(bqkv[:2048].reshape(1, 2048))
    bv_np = np.ascontiguousarray(bqkv[2048:].reshape(1, 1024))
    ones512_np = np.ones((1, 512), np.float32)
    ones16_np = np.ones((128, 16), np.float32)
    bp_rep = np.broadcast_to(bproj, (128, C)).copy()
    bp_zero = np.zeros((128, C), np.float32)

    in_maps = []
    for c in range(N_CORES):
        b, q = divmod(c, 4)
        xT_np = np.ascontiguousarray(
            x[b, RPC * q:RPC * (q + 1), :].reshape(RPC, 8, 128)
            .transpose(2, 1, 0))
        wp_np = np.ascontiguousarray(
            Wproj[256 * q:256 * (q + 1), :].reshape(2, 128, 1024)
            .transpose(1, 0, 2).reshape(128, 2048))
        in_maps.append({
            "xT": xT_np, "wq": wq_np, "wv": wv_np, "bqk": bqk_np,
            "bv": bv_np, "wp": wp_np,
            "bp": bp_rep if q == 0 else bp_zero,
            "ones512": ones512_np, "ones16": ones16_np,
        })
    return in_maps


def kernel(x, Wqkv, bqkv, Wproj, bproj):
    nc = _get_program()
    in_maps = _prep_core_inputs(x, Wqkv, bqkv, Wproj, bproj)
    res = run_bass_kernel_spmd(nc, in_maps, list(range(N_CORES)))
    out = np.zeros((B, T, C), dtype=np.float32)
    for c in range(N_CORES):
        out[c // 4] += res.results[c]["out"]
    return out
